# revision 32
# baseline (speedup 1.0000x reference)
"""MEX (log-mean-exp) 3x3 pooling kernel for Trainium2, 8-core data-parallel.

Math: out[n,i,h,w] = log( (1/K) * sum_{c,kh,kw} exp(x[n,c,h+kh-1,w+kw-1] + o[i,c,kh,kw]) )
with zero-padded x OOB (contributing exp(0+o) = exp(o)) and K = 32*3*3 = 288.

Factorization (EPS=1, f32 range is safe without max-subtraction):
    out = log( (1/K) * conv3x3( exp(xpad), exp(o) ) )
where exp(xpad) has 1.0 at padding (= exp(0)).

Per-core mapping (one image per core), single-load layout:
  - x is loaded ONCE into SBUF as [(j,c)=128, s=34, wp=132] f32 where j=0..3 is
    a 32-row group (rows 32j-1..32j+32 incl. halo) -> 2.1 MiB HBM traffic
    instead of 3x row-shifted copies.
  - One exp pass (f32 -> bf16) at full 128-partition width.
  - Weights are preprocessed HOST-side (np.exp of the 9216-element offsets)
    into 9 block-diagonal [128,128] bf16 matrices W[kh,kw][(j,c),(j,i)] =
    exp(o[i,c,kh,kw]) * delta_jj. One matmul then contracts c for all 4 row
    groups at once; the 9 (kh,kw) taps are free-dim offsets accumulated in
    PSUM. Each PSUM bank [128=(j,i), 512=(4 rows x 128 w)] takes 9 matmuls,
    then one wide Ln activation pass (scale=1/288) -> SBUF f32 -> HBM.
"""

import numpy as np
import ml_dtypes

import concourse.bacc as bacc
import concourse.tile as tile
import concourse.mybir as mybir
from concourse.bass_utils import run_bass_kernel_spmd

F32 = mybir.dt.float32
BF16 = mybir.dt.bfloat16
AF = mybir.ActivationFunctionType

# Exp and Ln both live in the "natural_log_exp_and_others" activation-table
# set, but the default per-function set choice puts them in different sets,
# so a kernel alternating Exp/Ln reloads ACT tables (~1.3 us each) on every
# switch. Restrict Exp/Ln to the combined set (keeping dict order, so the
# set ids walrus emits still match act_info.json) -> one table load total.
_COMBINED_ACT_SET = "natural_log_exp_and_others"


def _patch_act_tables():
    if getattr(bacc, "_mex_act_patch", False):
        return
    orig = bacc.get_activation_tables

    def patched(arch):
        tables = {k: set(v) for k, v in orig(arch).items()}
        if _COMBINED_ACT_SET in tables:
            for name, fns in tables.items():
                if name != _COMBINED_ACT_SET:
                    fns.discard(AF.Exp)
                    fns.discard(AF.Ln)
        return tables

    bacc.get_activation_tables = patched
    bacc._mex_act_patch = True


_patch_act_tables()

N, C, H, W = 8, 32, 128, 128
I = 32
K = C * 3 * 3          # 288
J = 4                  # row groups of 32 rows each
S = 34                 # slots per group: rows 32j-1 .. 32j+32 (halo incl.)
WP = 132               # padded plane width: wp=0 pad, 1..128 image, 129 pad
BANKS = 8              # PSUM banks; bank b covers rows 4b..4b+4 of every group


def _build(repeats: int = 1):
    nc = bacc.Bacc("TRN2", target_bir_lowering=False, debug=False)
    x = nc.dram_tensor("x", [C, H, W], F32, kind="ExternalInput").ap()
    w = nc.dram_tensor("w", [128, 3, 3, 128], BF16, kind="ExternalInput").ap()
    out = nc.dram_tensor("out", [I, H, W], F32, kind="ExternalOutput").ap()

    # HBM views: partition dim (j,c) for x, (j,i) for out
    xr = x.rearrange("c (j t) w -> j c t w", j=J)               # [4, 32, 32, 128]
    out_r = out.rearrange("i (j b r) w -> j i b r w", j=J, b=BANKS, r=4)

    with tile.TileContext(nc) as tc:
        with (
            tc.tile_pool(name="wt", bufs=1) as wtp,
            tc.tile_pool(name="xf", bufs=3) as xfp,
            tc.tile_pool(name="ef", bufs=3) as efp,
            tc.tile_pool(name="ps", bufs=1, space="PSUM") as psp,
            tc.tile_pool(name="ob", bufs=3) as obp,
        ):
            wt = wtp.tile([128, 3, 3, 128], BF16)
            nc.sync.dma_start(wt[:], w)
            # 8 fixed PSUM bank tiles, reused every repeat (accumulation
            # start/stop delimits each repeat's group; Tile adds WAR deps)
            pss = [psp.tile([128, 4, W], F32, name=f"ps{b}") for b in range(BANKS)]
            # software-pipelined emission: body k emits loads/exp/matmuls of
            # repeat k but the Ln+store of repeat k-1, so exp(k+1) is queued
            # on the scalar engine BEFORE Ln(k) and runs during matmuls of
            # rep k -- the PE never waits on the scalar FIFO.
            prev = None
            for _rep in range(repeats):
                prev = _emit_body(nc, xr, out_r, wt, xfp, efp, pss, obp, prev)
            _emit_tail(nc, out_r, obp, prev)
    nc.compile()
    return nc


def _emit_body(nc, xr, out_r, wt, xfp, efp, pss, obp, prev):
    # xf is PACKED (no column pad): HBM rows are contiguous across h, so each
    # per-j DMA moves 8 KB-contiguous runs on both sides (descriptor-optimal).
    xf = xfp.tile([128, S, W], F32)
    # pad rows (x=0 -> exp=1 handled by Exp of memset-0 rows)
    nc.vector.memset(xf[0:32, 0:1, :], 0.0)
    nc.vector.memset(xf[96:128, S - 1 : S, :], 0.0)
    # interior halos: slot 0 of j>=1 is image row 32j-1 (= group j-1, t=31);
    # slot 33 of j<=2 is image row 32j+32 (= group j+1, t=0)
    nc.sync.dma_start(xf[32:128, 0:1, :], xr[0:3, :, 31:32, :])
    nc.gpsimd.dma_start(xf[0:96, S - 1 : S, :], xr[1:4, :, 0:1, :])
    # main rows: eight 4-row DMAs, full 128 partitions each, alternating
    # across the two independent ~45 GB/s DGE paths (HWDGE via sync, SWDGE
    # via gpsimd) -- paths add, and finer splits keep more descriptors
    # outstanding per ring
    for q in range(8):
        t0 = 4 * q
        eng = nc.sync if q % 2 == 0 else nc.gpsimd
        eng.dma_start(xf[:, t0 + 1 : t0 + 5, :], xr[:, :, t0 : t0 + 4, :])

    # Ln + store of the PREVIOUS repeat, emitted BEFORE this repeat's exp:
    # its inputs (PSUM of rep k-1) are ready at body start, so on the scalar
    # FIFO it runs immediately, and the output stores enter both DMA rings
    # right behind this repeat's input loads -- in and out overlap fully.
    _emit_tail(nc, out_r, obp, prev)

    # ef IS padded: wp=0 / wp=129 must hold exp(0)=1.0 for the conv's zero-pad
    ef = efp.tile([128, S, WP], BF16)
    nc.vector.memset(ef[:, :, 0:1], 1.0)
    nc.vector.memset(ef[:, :, 129:WP], 1.0)
    # exp in 2 chunks aligned to the DMA halves (slot 17 lands with half 1);
    # packed f32 in -> padded bf16 out (strided write)
    nc.scalar.activation(ef[:, 0:17, 1:129], xf[:, 0:17], AF.Exp)
    nc.scalar.activation(ef[:, 17:S, 1:129], xf[:, 17:S], AF.Exp)

    # bank-major: 9 accumulating matmuls per PSUM bank
    for b in range(BANKS):
        for kh in range(3):
            for kw in range(3):
                nc.tensor.matmul(
                    pss[b][:],
                    wt[:, kh, kw, :],
                    ef[:, 4 * b + kh : 4 * b + kh + 4, kw : kw + W],
                    start=(kh == 0 and kw == 0),
                    stop=(kh == 2 and kw == 2),
                )
    return pss


def _emit_tail(nc, out_r, obp, prev):
    if prev is None:
        return
    ob = obp.tile([128, BANKS, 4, W], F32, name="ob")
    for b in range(BANKS):
        nc.scalar.activation(ob[:, b], prev[b][:], AF.Ln, scale=1.0 / K)
    # stores split across both DGE paths to balance them (each path carries
    # half the input + half the output, ~2.07 MiB per path per body), at
    # 2-bank granularity for more outstanding descriptors per ring
    nc.scalar.dma_start(out_r[:, :, 0:2], ob[:, 0:2])
    nc.gpsimd.dma_start(out_r[:, :, 2:4], ob[:, 2:4])
    nc.scalar.dma_start(out_r[:, :, 4:6], ob[:, 4:6])
    nc.gpsimd.dma_start(out_r[:, :, 6:8], ob[:, 6:8])


def _pack_weights(offsets: np.ndarray) -> np.ndarray:
    # host-side: exp + block-diagonal packing, [p=(j,c), kh, kw, q=(j,i)] bf16
    eo = np.exp(offsets.reshape(I, C, 3, 3).astype(np.float64)).astype(np.float32)
    wblk = np.zeros((128, 3, 3, 128), dtype=np.float32)
    for j in range(J):
        # wblk[32j+c, kh, kw, 32j+i] = exp(o[i, c, kh, kw])
        wblk[32 * j : 32 * j + 32, :, :, 32 * j : 32 * j + 32] = eo.transpose(
            1, 2, 3, 0
        )
    return wblk.astype(ml_dtypes.bfloat16)


_NC = None


def _get_nc():
    global _NC
    if _NC is None:
        _NC = _build()
    return _NC


def kernel(x: np.ndarray, offsets: np.ndarray) -> np.ndarray:
    x = np.ascontiguousarray(x, dtype=np.float32)
    wblk = _pack_weights(np.asarray(offsets, dtype=np.float32))
    nc = _get_nc()
    in_maps = [{"x": np.ascontiguousarray(x[i]), "w": wblk} for i in range(N)]
    res = run_bass_kernel_spmd(nc, in_maps, list(range(N))).results
    return np.stack([res[i]["out"] for i in range(N)], axis=0)


# revision 34
# speedup vs baseline: 1.1386x; 1.1386x over previous
"""MEX (log-mean-exp) 3x3 pooling kernel for Trainium2, 8-core data-parallel.

Math: out[n,i,h,w] = log( (1/K) * sum_{c,kh,kw} exp(x[n,c,h+kh-1,w+kw-1] + o[i,c,kh,kw]) )
with zero-padded x OOB (contributing exp(0+o) = exp(o)) and K = 32*3*3 = 288.

Factorization (EPS=1, f32 range is safe without max-subtraction):
    out = log( (1/K) * conv3x3( exp(xpad), exp(o) ) )
where exp(xpad) has 1.0 at padding (= exp(0)).

Per-core mapping (one image per core), single-load layout:
  - x is loaded ONCE into SBUF as [(j,c)=128, s=34, wp=132] f32 where j=0..3 is
    a 32-row group (rows 32j-1..32j+32 incl. halo) -> 2.1 MiB HBM traffic
    instead of 3x row-shifted copies.
  - One exp pass (f32 -> bf16) at full 128-partition width.
  - Weights are preprocessed HOST-side (np.exp of the 9216-element offsets)
    into 9 block-diagonal [128,128] bf16 matrices W[kh,kw][(j,c),(j,i)] =
    exp(o[i,c,kh,kw]) * delta_jj. One matmul then contracts c for all 4 row
    groups at once; the 9 (kh,kw) taps are free-dim offsets accumulated in
    PSUM. Each PSUM bank [128=(j,i), 512=(4 rows x 128 w)] takes 9 matmuls,
    then one wide Ln activation pass (scale=1/288) -> SBUF f32 -> HBM.
"""

import numpy as np
import ml_dtypes

import concourse.bacc as bacc
import concourse.tile as tile
import concourse.mybir as mybir
from concourse.bass_utils import run_bass_kernel_spmd

F32 = mybir.dt.float32
BF16 = mybir.dt.bfloat16
AF = mybir.ActivationFunctionType

# Exp and Ln both live in the "natural_log_exp_and_others" activation-table
# set, but the default per-function set choice puts them in different sets,
# so a kernel alternating Exp/Ln reloads ACT tables (~1.3 us each) on every
# switch. Restrict Exp/Ln to the combined set (keeping dict order, so the
# set ids walrus emits still match act_info.json) -> one table load total.
_COMBINED_ACT_SET = "natural_log_exp_and_others"


def _patch_act_tables():
    if getattr(bacc, "_mex_act_patch", False):
        return
    orig = bacc.get_activation_tables

    def patched(arch):
        tables = {k: set(v) for k, v in orig(arch).items()}
        if _COMBINED_ACT_SET in tables:
            for name, fns in tables.items():
                if name != _COMBINED_ACT_SET:
                    fns.discard(AF.Exp)
                    fns.discard(AF.Ln)
        return tables

    bacc.get_activation_tables = patched
    bacc._mex_act_patch = True


_patch_act_tables()

N, C, H, W = 8, 32, 128, 128
I = 32
K = C * 3 * 3          # 288
J = 4                  # row groups of 32 rows each
S = 34                 # slots per group: rows 32j-1 .. 32j+32 (halo incl.)
WP = 132               # padded plane width: wp=0 pad, 1..128 image, 129 pad
BANKS = 8              # PSUM banks; bank b covers rows 4b..4b+4 of every group


def _build(repeats: int = 1):
    nc = bacc.Bacc("TRN2", target_bir_lowering=False, debug=False)
    x = nc.dram_tensor("x", [C, H, W], F32, kind="ExternalInput").ap()
    w = nc.dram_tensor("w", [128, 3, 3, 128], BF16, kind="ExternalInput").ap()
    out = nc.dram_tensor("out", [I, H, W], F32, kind="ExternalOutput").ap()

    # HBM views: partition dim (j,c) for x, (j,i) for out
    xr = x.rearrange("c (j t) w -> j c t w", j=J)               # [4, 32, 32, 128]
    out_r = out.rearrange("i (j b r) w -> j i b r w", j=J, b=BANKS, r=4)

    with tile.TileContext(nc) as tc:
        with (
            tc.tile_pool(name="wt", bufs=1) as wtp,
            tc.tile_pool(name="xf", bufs=2) as xfp,
            tc.tile_pool(name="ef", bufs=2) as efp,
            tc.tile_pool(name="ps", bufs=1, space="PSUM") as psp,
            tc.tile_pool(name="ob", bufs=2) as obp,
        ):
            wt = wtp.tile([128, 3, 3, 128], BF16)
            nc.sync.dma_start(wt[:], w)
            # 8 fixed PSUM bank tiles, reused every repeat (accumulation
            # start/stop delimits each repeat's group; Tile adds WAR deps)
            pss = [psp.tile([128, 4, W], F32, name=f"ps{b}") for b in range(BANKS)]
            # software-pipelined emission: body k emits loads/exp/matmuls of
            # repeat k but the Ln+store of repeat k-1, so exp(k+1) is queued
            # on the scalar engine BEFORE Ln(k) and runs during matmuls of
            # rep k -- the PE never waits on the scalar FIFO.
            prev = None
            for _rep in range(repeats):
                prev = _emit_body(nc, xr, out_r, wt, xfp, efp, pss, obp, prev)
            _emit_tail(nc, out_r, obp, prev)
    nc.compile()
    return nc


def _emit_body(nc, xr, out_r, wt, xfp, efp, pss, obp, prev):
    # xf is PACKED (no column pad): HBM rows are contiguous across h, so each
    # per-j DMA moves 8 KB-contiguous runs on both sides (descriptor-optimal).
    xf = xfp.tile([128, S, W], F32)
    # pad rows (x=0 -> exp=1 handled by Exp of memset-0 rows)
    nc.vector.memset(xf[0:32, 0:1, :], 0.0)
    nc.vector.memset(xf[96:128, S - 1 : S, :], 0.0)
    # interior halos: slot 0 of j>=1 is image row 32j-1 (= group j-1, t=31);
    # slot 33 of j<=2 is image row 32j+32 (= group j+1, t=0)
    nc.sync.dma_start(xf[32:128, 0:1, :], xr[0:3, :, 31:32, :])
    nc.gpsimd.dma_start(xf[0:96, S - 1 : S, :], xr[1:4, :, 0:1, :])
    # main rows: four quarter-plane DMAs, full 128 partitions each,
    # alternating across the two independent ~45 GB/s DGE paths (HWDGE via
    # sync, SWDGE via gpsimd) -- paths add, and finer splits keep more
    # descriptors outstanding per ring (eighth-granularity measured worse:
    # SWDGE per-DMA fixed cost dominates below ~0.5 MiB)
    nc.sync.dma_start(xf[:, 1:9, :], xr[:, :, 0:8, :])
    nc.gpsimd.dma_start(xf[:, 9:17, :], xr[:, :, 8:16, :])
    nc.sync.dma_start(xf[:, 17:25, :], xr[:, :, 16:24, :])
    nc.gpsimd.dma_start(xf[:, 25:33, :], xr[:, :, 24:32, :])

    # Ln + store of the PREVIOUS repeat, emitted BEFORE this repeat's exp:
    # its inputs (PSUM of rep k-1) are ready at body start, so on the scalar
    # FIFO it runs immediately, and the output stores enter both DMA rings
    # right behind this repeat's input loads -- in and out overlap fully.
    _emit_tail(nc, out_r, obp, prev)

    # ef IS padded: wp=0 / wp=129 must hold exp(0)=1.0 for the conv's zero-pad
    ef = efp.tile([128, S, WP], BF16)
    nc.vector.memset(ef[:, :, 0:1], 1.0)
    nc.vector.memset(ef[:, :, 129:WP], 1.0)
    # exp in 2 chunks aligned to the DMA halves (slot 17 lands with half 1);
    # packed f32 in -> padded bf16 out (strided write)
    nc.scalar.activation(ef[:, 0:17, 1:129], xf[:, 0:17], AF.Exp)
    nc.scalar.activation(ef[:, 17:S, 1:129], xf[:, 17:S], AF.Exp)

    # bank-major: 9 accumulating matmuls per PSUM bank
    for b in range(BANKS):
        for kh in range(3):
            for kw in range(3):
                nc.tensor.matmul(
                    pss[b][:],
                    wt[:, kh, kw, :],
                    ef[:, 4 * b + kh : 4 * b + kh + 4, kw : kw + W],
                    start=(kh == 0 and kw == 0),
                    stop=(kh == 2 and kw == 2),
                )
    return pss


def _emit_tail(nc, out_r, obp, prev):
    if prev is None:
        return
    ob = obp.tile([128, BANKS, 4, W], F32, name="ob")
    for b in range(BANKS):
        nc.scalar.activation(ob[:, b], prev[b][:], AF.Ln, scale=1.0 / K)
    # stores split across both DGE paths to balance them (each path carries
    # half the input + half the output, ~2.07 MiB per path per body), at
    # 2-bank granularity for more outstanding descriptors per ring
    nc.scalar.dma_start(out_r[:, :, 0:2], ob[:, 0:2])
    nc.gpsimd.dma_start(out_r[:, :, 2:4], ob[:, 2:4])
    nc.scalar.dma_start(out_r[:, :, 4:6], ob[:, 4:6])
    nc.gpsimd.dma_start(out_r[:, :, 6:8], ob[:, 6:8])


def _pack_weights(offsets: np.ndarray) -> np.ndarray:
    # host-side: exp + block-diagonal packing, [p=(j,c), kh, kw, q=(j,i)] bf16
    eo = np.exp(offsets.reshape(I, C, 3, 3).astype(np.float64)).astype(np.float32)
    wblk = np.zeros((128, 3, 3, 128), dtype=np.float32)
    for j in range(J):
        # wblk[32j+c, kh, kw, 32j+i] = exp(o[i, c, kh, kw])
        wblk[32 * j : 32 * j + 32, :, :, 32 * j : 32 * j + 32] = eo.transpose(
            1, 2, 3, 0
        )
    return wblk.astype(ml_dtypes.bfloat16)


_NC = None


def _get_nc():
    global _NC
    if _NC is None:
        _NC = _build()
    return _NC


def kernel(x: np.ndarray, offsets: np.ndarray) -> np.ndarray:
    x = np.ascontiguousarray(x, dtype=np.float32)
    wblk = _pack_weights(np.asarray(offsets, dtype=np.float32))
    nc = _get_nc()
    in_maps = [{"x": np.ascontiguousarray(x[i]), "w": wblk} for i in range(N)]
    res = run_bass_kernel_spmd(nc, in_maps, list(range(N))).results
    return np.stack([res[i]["out"] for i in range(N)], axis=0)
